# revision 8
# baseline (speedup 1.0000x reference)
"""Trainium2 Bass kernel for the Deepeucloss loss function.

Computes a scalar loss from five [16, 128, 4096, 3] f32 tensors plus three
scalars.  Data-parallel across 8 NeuronCores: each core takes 2 of the 16
batches; the host combines the per-core partial statistics in float64 (the
all-reduce of the sharding hint).

Architecture (all decisions measured on HW via repetition-delta timing):

 - fp8(e4m3) inputs.  The 2e-2 rel-err budget dwarfs fp8 rounding: the
   end-to-end error is a deterministic ~1.8e-3 (inputs are fixed-seed).
   This cuts the HBM traffic that bounds every fp32/fp16 variant to
   15.75 MiB/core (f32: 60 MiB @ ~166 us; fp16: 30 MiB @ ~83 us).
 - Transposed upload layout [48, 128nd, 2e, 2b*128pt] per core so the
   (n,d)-axis lies on SBUF partitions (rows of 512 B keep DMA descriptors
   at full rate).  nd = c*256 + e*128 + p.
 - NO on-device subtractions (fp8 DVE subs run ~1x and bottleneck).
   Instead the squared-distance sums are expanded on the idle PE array:
     sum_nd (m - t)^2 = diag(G_m) - 2 diag(C_mt) + diag(G_t)
   where G/C are PSUM-accumulated Gram/cross matmuls of the raw input
   tiles.  DoubleRow fp8 perf mode contracts the 2 nd-blocks of the
   e-axis in one instruction at 0.5 cycles/row: 8 matmuls per
   (k-block, batch), ~20 us of PE time for the whole pass.
 - DVE does only the 16 diagonal extractions (Gram * identity with
   fused row-sum); ACT does only ln(gt2_var) for the KL base term.
 - 1 MiB DMAs (KG=16 nd-pair blocks per tile), io_bufs=4: in-flight
   DMAs keep the DMA engines fed; smaller or larger DMAs, splitting
   across HWDGE queues (SP/ACT/GPSIMD), bigger descriptors (E=4/16
   layouts), and a fully-linear DRAM-==-SBUF layout all measured equal
   or slower, so ~68-70 us appears to be a platform fp8-path ceiling.

Math (NUM_CLASSES=128, L2_LAMBDA=0.01, S2=2.0):
  euc(m)   = sum_{b,p} sqrt(sum_{n,d} (m - target)^2) / 128
  base     = log(2/s1) + s1^2/8 - 0.5          (s1 = gt2_var)
  kl       = 1.4*sum(base) + (S0 + 0.2*S1 + 0.2*S2)/8,
             Sk = sum((m_k - target)^2)
  outloss  = euc(out) + 0.002*l_dynamic*leg
  gt_loss  = 0.1*euc(gt1_mean) + 0.2*euc(gt2_mean)
  reg      = gt0 * 0.01 * l_dynamic
  result   = outloss + gt_loss + reg + kl / (1.2*(euc(out) + gt_loss))

Per-core output: stats [128, 19] f32 = 16 diagonal columns (8 Gram/cross
slots x 2 batches) + 3 per-group ln partial columns.

Measured: ~68-70 us/pass (best uncontended windows, paired high-rep
deltas) vs 166 us for the f32 streaming baseline; ~2.4x.  The shared
device alternates between contended and uncontended windows, so timing
is min-based over many rounds.
"""

from contextlib import nullcontext

import numpy as np
import ml_dtypes

import concourse.bacc as bacc
import concourse.tile as tile
import concourse.mybir as mybir
from concourse import bass_utils

B, P, N, D = 16, 128, 4096, 3
F = N * D                      # 12288 elements per (batch, point) row
NCORES = 8
BL = B // NCORES               # batches per core
CB2 = F // (2 * P)             # 48 nd-pair blocks (2 x 128 nd each)
KG = 16                        # nd-pair blocks per io tile (1 MiB DMAs)
NG = CB2 // KG                 # 3 groups per pass
NSTAT = 16 + NG                # 16 diag cols + NG ln cols
CORE_IDS = list(range(NCORES))

IN_NAMES = ("t_out", "t_tgt", "t_gt1", "t_gt2", "t_s1")
# PSUM slots: Grams of out,tgt,gt1,gt2; crosses out.tgt, gt1.tgt, gt2.tgt;
# Gram of s1.  diag(S_m) is reassembled on the host.
GRAMS = (("t_out", "t_out"), ("t_tgt", "t_tgt"), ("t_gt1", "t_gt1"),
         ("t_gt2", "t_gt2"), ("t_out", "t_tgt"), ("t_gt1", "t_tgt"),
         ("t_gt2", "t_tgt"), ("t_s1", "t_s1"))

_CACHE = {}
LAST_RESULTS = None            # BassKernelResults of the most recent run


def _build(reps=1):
    # reps>1 wraps the streaming loop in a hardware For_i (same result; every
    # repetition recomputes the same stats) — used only for repetition-delta
    # timing in test.py.  The graded path always builds with reps=1.
    fp16 = mybir.dt.float16
    fp8e4 = mybir.dt.float8e4
    fp32 = mybir.dt.float32
    Ln = mybir.ActivationFunctionType.Ln
    Mult = mybir.AluOpType.mult
    DR = mybir.MatmulPerfMode.DoubleRow

    nc = bacc.Bacc(
        "TRN2", target_bir_lowering=False, debug=False, num_devices=NCORES
    )
    ins = {
        n: nc.dram_tensor(n, [CB2, P, 2, 2 * P], fp8e4, kind="ExternalInput").ap()
        for n in IN_NAMES
    }
    ident = nc.dram_tensor("ident", [P, P], fp32, kind="ExternalInput").ap()
    stats = nc.dram_tensor("stats", [P, NSTAT], fp32, kind="ExternalOutput").ap()
    tshape = [P, KG, 2, 2 * P]

    with tile.TileContext(nc) as tc:
        with (
            tc.tile_pool(name="io", bufs=4) as io_pool,
            tc.tile_pool(name="scr", bufs=1) as scr_pool,
            tc.tile_pool(name="acc", bufs=1) as acc_pool,
            tc.psum_pool(name="ps", bufs=1) as ps_pool,
        ):
            acc = acc_pool.tile([P, NSTAT], fp32, tag="acc", name="acc")
            scr_ln = scr_pool.tile(tshape, fp16, tag="scr_ln", name="scr_ln")
            scr_dg = scr_pool.tile([P, P], fp32, tag="scr_dg", name="scr_dg")
            id_t = scr_pool.tile([P, P], fp32, tag="id", name="id")
            nc.sync.dma_start(id_t[:], ident)

            # one PSUM bank per slot; the two batches share it column-wise
            psums = {
                j: ps_pool.tile([P, BL, P], fp32, tag=f"ps{j}", name=f"ps{j}")
                for j in range(8)
            }

            rep_loop = tc.For_i(0, reps, 1) if reps > 1 else nullcontext()
            with rep_loop:
                for g in range(NG):
                    tl = {}
                    for n in IN_NAMES:
                        tl[n] = io_pool.tile(tshape, fp8e4, tag=n, name=n)
                        src = ins[n][g * KG:(g + 1) * KG]
                        nc.sync.dma_start(tl[n][:], src.transpose([1, 0, 2, 3]))

                    nc.scalar.activation(
                        scr_ln[:], tl["t_s1"][:], Ln,
                        accum_out=acc[:, 16 + g: 17 + g],
                    )

                    for k in range(KG):
                        for b in range(BL):
                            for j, (lhs, rhs) in enumerate(GRAMS):
                                xl = tl[lhs][:, k, :, b * P:(b + 1) * P]
                                xr = tl[rhs][:, k, :, b * P:(b + 1) * P]
                                nc.tensor.matmul(
                                    psums[j][:, b, :], xl, xr,
                                    start=(g == 0 and k == 0),
                                    stop=(g == NG - 1 and k == KG - 1),
                                    perf_mode=DR, skip_group_check=True,
                                )

                # acc[:, j*BL+b] = diag(G_jb) via Gram x identity + row-sum
                for j in range(8):
                    for b in range(BL):
                        nc.vector.scalar_tensor_tensor(
                            scr_dg[:], psums[j][:, b, :], 1.0, id_t[:],
                            Mult, Mult,
                            accum_out=acc[:, j * BL + b: j * BL + b + 1],
                        )

            nc.sync.dma_start(stats, acc[:])
    nc.compile()
    return nc


def _get_nc():
    if "nc" not in _CACHE:
        _CACHE["nc"] = _build()
    return _CACHE["nc"]


def make_in_maps(out, target, gt1_mean, gt2_mean, gt2_var):
    """e4m3-cast, batch-shard, transpose to [c, nd, e, (b,pt)] per core."""
    full = {
        "t_out": out, "t_tgt": target, "t_gt1": gt1_mean,
        "t_gt2": gt2_mean, "t_s1": gt2_var,
    }
    ident = np.eye(P, dtype=np.float32)
    in_maps = [{"ident": ident} for _ in CORE_IDS]
    for name, a in full.items():
        a8 = (np.asarray(a, np.float32)
              .astype(ml_dtypes.float8_e4m3).reshape(B, P, F))
        for i in CORE_IDS:
            sl = a8[i * BL:(i + 1) * BL]                  # [2, 128, 12288]
            t = (sl.reshape(BL, P, CB2, 2, P)             # nd = c*256+e*128+p
                 .transpose(2, 4, 3, 0, 1)                # [c, p, e, b, pt]
                 .reshape(CB2, P, 2, 2 * P))
            in_maps[i][name] = np.ascontiguousarray(t)
    return in_maps


def _stats_sane(stats):
    """Guard against a rare transient bad dispatch (seen once): stats must be
    finite, Gram diagonals positive, and per-core totals consistent (inputs
    are iid across the batch shard, so core sums agree to a few percent)."""
    if not np.isfinite(stats).all():
        return False
    dg = stats[:, :, :16].reshape(NCORES, P, 8, BL)
    if (dg[:, :, (0, 1, 2, 3, 7)] <= 0.0).any():
        return False
    core_tot = dg[:, :, 0].sum(axis=(1, 2))              # per-core G_out mass
    return core_tot.min() > 0.6 * core_tot.max()


def kernel(out, target, gt0, gt1_mean, gt2_mean, gt2_var, leg, l_dynamic):
    global LAST_RESULTS
    nc = _get_nc()

    in_maps = make_in_maps(out, target, gt1_mean, gt2_mean, gt2_var)
    res = bass_utils.run_bass_kernel_spmd(nc, in_maps, CORE_IDS)
    LAST_RESULTS = res

    stats = np.stack(
        [np.asarray(r["stats"], dtype=np.float64) for r in res.results]
    )                                                     # [8, P, NSTAT]
    if not _stats_sane(stats):
        res = bass_utils.run_bass_kernel_spmd(nc, in_maps, CORE_IDS)
        LAST_RESULTS = res
        stats = np.stack(
            [np.asarray(r["stats"], dtype=np.float64) for r in res.results]
        )
    dg = stats[:, :, :16].reshape(NCORES, P, 8, BL)       # [core, pt, j, b]
    # S_m = G_m - 2 C_mt + G_t  per (core, point, batch)
    S = [dg[:, :, jm] - 2.0 * dg[:, :, jc] + dg[:, :, 1]
         for jm, jc in ((0, 4), (2, 5), (3, 6))]
    euc0, euc1, euc2 = (np.sqrt(np.maximum(Sk, 0.0)).sum() / 128.0 for Sk in S)
    s0, s1, s2 = (Sk.sum() for Sk in S)
    sq_sum = dg[:, :, 7].sum()
    ln_sum = stats[:, :, 16:].sum()

    ntot = float(B * P * F)
    base_sum = ntot * np.log(2.0) - ln_sum + sq_sum / 8.0 - 0.5 * ntot
    kl = 1.4 * base_sum + (s0 + 0.2 * s1 + 0.2 * s2) / 8.0

    l_dyn, leg_v, gt0_v = float(l_dynamic), float(leg), float(gt0)
    outloss = euc0 + 0.01 * 0.2 * l_dyn * leg_v
    gt_loss = 0.1 * euc1 + 0.2 * euc2
    reg = gt0_v * 0.01 * l_dyn
    result = outloss + gt_loss + reg + kl / (1.2 * (euc0 + gt_loss))
    return np.asarray(result, dtype=np.float32)


# revision 9
# speedup vs baseline: 1.4979x; 1.4979x over previous
"""Trainium2 Bass kernel for the Deepeucloss loss function.

Computes a scalar loss from five [16, 128, 4096, 3] f32 tensors plus three
scalars.  Data-parallel across 8 NeuronCores: each core takes 2 of the 16
batches; the host combines the per-core partial statistics in float64 (the
all-reduce of the sharding hint).

Architecture (all decisions measured on HW via repetition-delta timing):

 - fp8(e4m3) inputs.  The 2e-2 rel-err budget dwarfs fp8 rounding: the
   end-to-end error is a deterministic ~1.8e-3 (inputs are fixed-seed).
   This cuts the HBM traffic that bounds every fp32/fp16 variant to
   15.75 MiB/core (f32: 60 MiB @ ~166 us; fp16: 30 MiB @ ~83 us).
 - Transposed upload layout [48, 128nd, 2e, 2b*128pt] per core so the
   (n,d)-axis lies on SBUF partitions (rows of 512 B keep DMA descriptors
   at full rate).  nd = c*256 + e*128 + p.
 - NO on-device subtractions (fp8 DVE subs run ~1x and bottleneck).
   Instead the squared-distance sums are expanded on the idle PE array:
     sum_nd (m - t)^2 = diag(G_m) - 2 diag(C_mt) + diag(G_t)
   where G/C are PSUM-accumulated Gram/cross matmuls of the raw input
   tiles.  DoubleRow fp8 perf mode contracts the 2 nd-blocks of the
   e-axis in one instruction at 0.5 cycles/row: 8 matmuls per
   (k-block, batch), ~20 us of PE time for the whole pass.
 - DVE does only the 16 diagonal extractions (Gram * identity with
   fused row-sum); ACT does only ln(gt2_var) for the KL base term.
 - 1 MiB DMAs (KG=16 nd-pair blocks per tile), io_bufs=4: in-flight
   DMAs keep the DMA engines fed; smaller or larger DMAs, splitting
   across HWDGE queues (SP/ACT/GPSIMD), bigger descriptors (E=4/16
   layouts), and a fully-linear DRAM-==-SBUF layout all measured equal
   or slower, so ~68-70 us appears to be a platform fp8-path ceiling.

Math (NUM_CLASSES=128, L2_LAMBDA=0.01, S2=2.0):
  euc(m)   = sum_{b,p} sqrt(sum_{n,d} (m - target)^2) / 128
  base     = log(2/s1) + s1^2/8 - 0.5          (s1 = gt2_var)
  kl       = 1.4*sum(base) + (S0 + 0.2*S1 + 0.2*S2)/8,
             Sk = sum((m_k - target)^2)
  outloss  = euc(out) + 0.002*l_dynamic*leg
  gt_loss  = 0.1*euc(gt1_mean) + 0.2*euc(gt2_mean)
  reg      = gt0 * 0.01 * l_dynamic
  result   = outloss + gt_loss + reg + kl / (1.2*(euc(out) + gt_loss))

Per-core output: stats [128, 19] f32 = 16 diagonal columns (8 Gram/cross
slots x 2 batches) + 3 per-group ln partial columns.

Measured: ~68-70 us/pass (best uncontended windows, paired high-rep
deltas) vs 166 us for the f32 streaming baseline; ~2.4x.  The shared
device alternates between contended and uncontended windows, so timing
is min-based over many rounds.
"""

from contextlib import nullcontext

import numpy as np
import ml_dtypes

import concourse.bacc as bacc
import concourse.tile as tile
import concourse.mybir as mybir
from concourse import bass_utils

B, P, N, D = 16, 128, 4096, 3
F = N * D                      # 12288 elements per (batch, point) row
NCORES = 8
BL = B // NCORES               # batches per core
CB2 = F // (2 * P)             # 48 nd-pair blocks (2 x 128 nd each)
KG = 16                        # nd-pair blocks per io tile (1 MiB DMAs)
NG = CB2 // KG                 # 3 groups per pass
NSTAT = 16 + NG                # 16 diag cols + NG ln cols
CORE_IDS = list(range(NCORES))

IN_NAMES = ("t_out", "t_tgt", "t_gt1", "t_gt2", "t_s1")
# PSUM slots: Grams of out,tgt,gt1,gt2; crosses out.tgt, gt1.tgt, gt2.tgt;
# Gram of s1.  diag(S_m) is reassembled on the host.
GRAMS = (("t_out", "t_out"), ("t_tgt", "t_tgt"), ("t_gt1", "t_gt1"),
         ("t_gt2", "t_gt2"), ("t_out", "t_tgt"), ("t_gt1", "t_tgt"),
         ("t_gt2", "t_tgt"), ("t_s1", "t_s1"))

_CACHE = {}
LAST_RESULTS = None            # BassKernelResults of the most recent run


def _build(reps=1):
    # reps>1 wraps the streaming loop in a hardware For_i (same result; every
    # repetition recomputes the same stats) — used only for repetition-delta
    # timing in test.py.  The graded path always builds with reps=1.
    fp16 = mybir.dt.float16
    fp8e4 = mybir.dt.float8e4
    fp32 = mybir.dt.float32
    Ln = mybir.ActivationFunctionType.Ln
    Mult = mybir.AluOpType.mult
    DR = mybir.MatmulPerfMode.DoubleRow

    nc = bacc.Bacc(
        "TRN2", target_bir_lowering=False, debug=False, num_devices=NCORES
    )
    ins = {
        n: nc.dram_tensor(n, [CB2, P, 2, 2 * P], fp8e4, kind="ExternalInput").ap()
        for n in IN_NAMES
    }
    ident = nc.dram_tensor("ident", [P, P], fp32, kind="ExternalInput").ap()
    stats = nc.dram_tensor("stats", [P, NSTAT], fp32, kind="ExternalOutput").ap()
    tshape = [P, KG, 2, 2 * P]

    with tile.TileContext(nc) as tc:
        with (
            tc.tile_pool(name="io", bufs=4) as io_pool,
            tc.tile_pool(name="scr", bufs=1) as scr_pool,
            tc.tile_pool(name="acc", bufs=1) as acc_pool,
            tc.psum_pool(name="ps", bufs=1) as ps_pool,
        ):
            acc = acc_pool.tile([P, NSTAT], fp32, tag="acc", name="acc")
            # fp8 scratch for ln's discarded elementwise output: halves the
            # SBUF write traffic that contends with DMA/PE (ln range fits)
            scr_ln = scr_pool.tile(tshape, fp8e4, tag="scr_ln", name="scr_ln")
            scr_dg = scr_pool.tile([P, P], fp32, tag="scr_dg", name="scr_dg")
            id_t = scr_pool.tile([P, P], fp32, tag="id", name="id")
            nc.sync.dma_start(id_t[:], ident)

            # one PSUM bank per slot; the two batches share it column-wise
            psums = {
                j: ps_pool.tile([P, BL, P], fp32, tag=f"ps{j}", name=f"ps{j}")
                for j in range(8)
            }

            rep_loop = tc.For_i(0, reps, 1) if reps > 1 else nullcontext()
            with rep_loop:
                for g in range(NG):
                    tl = {}
                    for n in IN_NAMES:
                        tl[n] = io_pool.tile(tshape, fp8e4, tag=n, name=n)
                        src = ins[n][g * KG:(g + 1) * KG]
                        nc.sync.dma_start(tl[n][:], src.transpose([1, 0, 2, 3]))

                    nc.scalar.activation(
                        scr_ln[:], tl["t_s1"][:], Ln,
                        accum_out=acc[:, 16 + g: 17 + g],
                    )

                    for k in range(KG):
                        for b in range(BL):
                            for j, (lhs, rhs) in enumerate(GRAMS):
                                xl = tl[lhs][:, k, :, b * P:(b + 1) * P]
                                xr = tl[rhs][:, k, :, b * P:(b + 1) * P]
                                nc.tensor.matmul(
                                    psums[j][:, b, :], xl, xr,
                                    start=(g == 0 and k == 0),
                                    stop=(g == NG - 1 and k == KG - 1),
                                    perf_mode=DR, skip_group_check=True,
                                )

                # acc[:, j*BL+b] = diag(G_jb) via Gram x identity + row-sum
                for j in range(8):
                    for b in range(BL):
                        nc.vector.scalar_tensor_tensor(
                            scr_dg[:], psums[j][:, b, :], 1.0, id_t[:],
                            Mult, Mult,
                            accum_out=acc[:, j * BL + b: j * BL + b + 1],
                        )

            nc.sync.dma_start(stats, acc[:])
    nc.compile()
    return nc


def _get_nc():
    if "nc" not in _CACHE:
        _CACHE["nc"] = _build()
    return _CACHE["nc"]


def make_in_maps(out, target, gt1_mean, gt2_mean, gt2_var):
    """e4m3-cast, batch-shard, transpose to [c, nd, e, (b,pt)] per core."""
    full = {
        "t_out": out, "t_tgt": target, "t_gt1": gt1_mean,
        "t_gt2": gt2_mean, "t_s1": gt2_var,
    }
    ident = np.eye(P, dtype=np.float32)
    in_maps = [{"ident": ident} for _ in CORE_IDS]
    for name, a in full.items():
        a8 = (np.asarray(a, np.float32)
              .astype(ml_dtypes.float8_e4m3).reshape(B, P, F))
        for i in CORE_IDS:
            sl = a8[i * BL:(i + 1) * BL]                  # [2, 128, 12288]
            t = (sl.reshape(BL, P, CB2, 2, P)             # nd = c*256+e*128+p
                 .transpose(2, 4, 3, 0, 1)                # [c, p, e, b, pt]
                 .reshape(CB2, P, 2, 2 * P))
            in_maps[i][name] = np.ascontiguousarray(t)
    return in_maps


def _stats_sane(stats):
    """Guard against a rare transient bad dispatch (seen once): stats must be
    finite, Gram diagonals positive, and per-core totals consistent (inputs
    are iid across the batch shard, so core sums agree to a few percent)."""
    if not np.isfinite(stats).all():
        return False
    dg = stats[:, :, :16].reshape(NCORES, P, 8, BL)
    if (dg[:, :, (0, 1, 2, 3, 7)] <= 0.0).any():
        return False
    core_tot = dg[:, :, 0].sum(axis=(1, 2))              # per-core G_out mass
    return core_tot.min() > 0.6 * core_tot.max()


def kernel(out, target, gt0, gt1_mean, gt2_mean, gt2_var, leg, l_dynamic):
    global LAST_RESULTS
    nc = _get_nc()

    in_maps = make_in_maps(out, target, gt1_mean, gt2_mean, gt2_var)
    res = bass_utils.run_bass_kernel_spmd(nc, in_maps, CORE_IDS)
    LAST_RESULTS = res

    stats = np.stack(
        [np.asarray(r["stats"], dtype=np.float64) for r in res.results]
    )                                                     # [8, P, NSTAT]
    if not _stats_sane(stats):
        res = bass_utils.run_bass_kernel_spmd(nc, in_maps, CORE_IDS)
        LAST_RESULTS = res
        stats = np.stack(
            [np.asarray(r["stats"], dtype=np.float64) for r in res.results]
        )
    dg = stats[:, :, :16].reshape(NCORES, P, 8, BL)       # [core, pt, j, b]
    # S_m = G_m - 2 C_mt + G_t  per (core, point, batch)
    S = [dg[:, :, jm] - 2.0 * dg[:, :, jc] + dg[:, :, 1]
         for jm, jc in ((0, 4), (2, 5), (3, 6))]
    euc0, euc1, euc2 = (np.sqrt(np.maximum(Sk, 0.0)).sum() / 128.0 for Sk in S)
    s0, s1, s2 = (Sk.sum() for Sk in S)
    sq_sum = dg[:, :, 7].sum()
    ln_sum = stats[:, :, 16:].sum()

    ntot = float(B * P * F)
    base_sum = ntot * np.log(2.0) - ln_sum + sq_sum / 8.0 - 0.5 * ntot
    kl = 1.4 * base_sum + (s0 + 0.2 * s1 + 0.2 * s2) / 8.0

    l_dyn, leg_v, gt0_v = float(l_dynamic), float(leg), float(gt0)
    outloss = euc0 + 0.01 * 0.2 * l_dyn * leg_v
    gt_loss = 0.1 * euc1 + 0.2 * euc2
    reg = gt0_v * 0.01 * l_dyn
    result = outloss + gt_loss + reg + kl / (1.2 * (euc0 + gt_loss))
    return np.asarray(result, dtype=np.float32)
